# revision 23
# baseline (speedup 1.0000x reference)
"""Trainium2 Bass kernel for nn_CoNe_35974646071945 (retrieval_knn).

Strategy: K-shard the 65536-entry queue across 8 NeuronCores.
Per core (shard of KS=8192 queue columns):
  pq:  sim8[b, j] = norm_q @ queue_shard as one fp8 DoubleRow matmul per
       (b-tile, j-chunk) (256-deep contraction per pass), shipped fp8.
       The host then refines: exact fp64 sims for the top-1024 fp8
       candidates per row (the true top-200 sit within fp8-rank ~570 on
       this distribution; 1024 gives 2x margin), so the top-200
       selection and softmax weights are exact.
  pk:  simk^T[j, b] via one fp8 DoubleRow matmul per 128-j tile, then
       ET = exp(10*simk) on the scalar engine, stored fp8 in SBUF.
  p2:  P[b, c'] = ET^T @ qlp_aug (fp8 DoubleRow, 64x-scaled
       stochastically-rounded qlp so the fp8 rounding is unbiased;
       col 1000 = 64 gives the softmax partition Z). PSUM groups are
       512-column (bank-aligned) so the bank-granular start-zeroing
       cannot clobber a sibling accumulation group.
Host: exact top-200/softmax for supcon, fc loss, and the KL with
dc_t = P[:, :1000]/P[:, 1000] (the 64x scale cancels).
"""
import sys
sys.path.insert(0, '/opt/trn_rl_repo')
sys.path.insert(0, '/root/.axon_site/_ro/trn_rl_repo')

import numpy as np
import ml_dtypes
from contextlib import ExitStack

from concourse import bass, tile, mybir
from concourse.bass_utils import run_bass_kernel_spmd
from concourse.vector_clock import ScopedClock, VectorClock

F32 = mybir.dt.float32
F16 = mybir.dt.float16
FP8 = mybir.dt.float8e4
Act = mybir.ActivationFunctionType
PM = mybir.MatmulPerfMode
FP8NP = ml_dtypes.float8_e4m3

N_CORES = 8
B, D, K, C = 512, 256, 65536, 1000
KS = K // N_CORES            # 8192 queue columns per core
T_SUP, T_DC, LS = 0.07, 0.1, 0.1
EPS = 1e-8
NJT = KS // 128              # 64 j-tiles per core
NS = NJT // 2                # 32 double-j-tiles (fp8 DoubleRow)
NJC = KS // 512              # 16 j-chunks for pq
CP = 1024                    # padded class dim (1000 probs + Z col + pad)
QLP_SCALE = 64.0
CAND = 1024                  # fp8 candidates refined per row on host


class CompatTileContext(tile.TileContext):
    """This walrus build encodes at most ONE sync wait per instruction.
    Split Tile's multi-wait instructions and its tail drain."""

    def _commit_instruction(self, inst, lazy_reg_writes=True):
        si = inst.sync_info
        if (
            si is not None
            and si.on_wait
            and len(si.on_wait) > 1
            and inst.engine != mybir.EngineType.Unassigned
        ):
            import bass_rust
            waits = list(si.on_wait)
            for w in waits[:-1]:
                nop = mybir.InstNoOp(
                    name=f"I-{self.nc.next_id()}", ins=[], outs=[]
                )
                nop.engine = inst.engine
                nop.sync_info = bass_rust.SyncInfo(on_wait=[w], on_update=[])
                super()._commit_instruction(nop, lazy_reg_writes=False)
            si.on_wait = [waits[-1]]
            inst.sync_info = si
        super()._commit_instruction(inst, lazy_reg_writes=lazy_reg_writes)

    def _drain_and_barrier(self, tick_clock, wait_clock):
        gclock = tick_clock.global_clock
        n = len(gclock)
        for i in range(n):
            if gclock[i] == 0:
                continue
            vec = [0] * n
            vec[i] = gclock[i]
            nop_inst = self.nc.sync.nop(nofuse=True, hint=f"tail_wait_p{i}")
            wait_clock.add_sem_waits(
                nop_inst.ins, ScopedClock({None: VectorClock(vec)})
            )
        self.nc.sync.drain()
        self.nc.all_engine_barrier()
        assert self.sems is not None
        popped = self.nc._tile_sem_poison_stack.pop()
        assert popped is self._sem_poison
        self.nc.clear_and_free_semaphores(list(self.sems.allocated().values()))
        self.nc.all_engine_barrier()


_CACHED = {}


def _p2a(nc, pacc, et3, qlb, jc):
    """Phase-2 pass A matmuls for the s-pair of j-chunk jc (b-tiles 0,1)."""
    for si in range(2):
        s = 2 * jc + si
        for bt in range(2):
            lhs = et3[:, 2 * s:2 * s + 2, bt * 128:(bt + 1) * 128]
            for c0, c1 in ((0, 512), (512, 1001)):
                nc.tensor.matmul(
                    pacc[bt][:, c0:c1],
                    lhs,
                    qlb[:, 2 * s:2 * s + 2, c0:c1],
                    start=(s == 0), stop=(s == NS - 1),
                    perf_mode=PM.DoubleRow)


def _build():
    if 'nc' in _CACHED:
        return _CACHED['nc']
    nc = bass.Bass(num_devices=N_CORES)
    qT8_in = nc.declare_dram_parameter("qT8", [128, 2, B], FP8, isOutput=False)
    kT8_in = nc.declare_dram_parameter("kT8", [128, 2, B], FP8, isOutput=False)
    qsh8_in = nc.declare_dram_parameter("qsh8", [128, 2, KS], FP8, isOutput=False)
    # host layout [p][s][h][c] flattened to [128, NS*2*CP]
    qlp8_in = nc.declare_dram_parameter("qlp8", [128, NS * 2 * CP], FP8,
                                        isOutput=False)
    simw_out = nc.declare_dram_parameter("simw", [B, KS], FP8, isOutput=True)
    p_out = nc.declare_dram_parameter("pout", [B, CP], F32, isOutput=True)

    with ExitStack() as ctx:
        tc = ctx.enter_context(CompatTileContext(nc))
        pool = ctx.enter_context(tc.tile_pool(name="main", bufs=1))
        sq = ctx.enter_context(tc.tile_pool(name="sq", bufs=2))

        qT8 = pool.tile([128, 2, B], FP8, name="qT8")
        kT8 = pool.tile([128, 2, B], FP8, name="kT8")
        qsh8 = pool.tile([128, 2, KS], FP8, name="qsh8")
        et3 = pool.tile([128, NJT, B], FP8, name="et3")
        qlb = pool.tile([128, NS * 2, CP], FP8, name="qlb")

        # head: h-granular so the first pq/pk matmuls start ASAP
        for h in range(2):
            nc.sync.dma_start(qT8[:, h, :], qT8_in[:, h, :])
            nc.sync.dma_start(qsh8[:, h, 0:512], qsh8_in[:, h, 0:512])
        nc.sync.dma_start(qsh8[:, :, 512:1024], qsh8_in[:, :, 512:1024])
        for h in range(2):
            nc.sync.dma_start(kT8[:, h, :], kT8_in[:, h, :])
        for u in range(8):
            nc.sync.dma_start(qlb[:, u:u + 1, :],
                              qlp8_in[:, u * CP:(u + 1) * CP])

        warm8 = pool.tile([128, 512], FP8, name="warm8")
        nc.vector.memset(warm8[:], 0.0)

        with ExitStack() as ph:
            psA = ph.enter_context(tc.tile_pool(name="psA", bufs=1, space="PSUM"))
            pacc = [psA.tile([128, CP], F32, name=f"pacc{i}") for i in range(2)]
            with ExitStack() as ph1:
                psm = ph1.enter_context(
                    tc.tile_pool(name="psm", bufs=2, space="PSUM"))
                st_cur = {}
                # ramp the PE to full p-state while the head DMAs land
                for _ in range(7):
                    wp = psm.tile([128, 512], F32, name="wp", tag="pk")
                    nc.tensor.matmul(wp[0:64, :], warm8[:, 0:64], warm8[:],
                                     start=True, stop=True)
                for jc in range(NJC):
                    jsl = slice(jc * 512, (jc + 1) * 512)
                    if jc + 2 < NJC:
                        nsl = slice((jc + 2) * 512, (jc + 3) * 512)
                        nc.sync.dma_start(qsh8[:, :, nsl], qsh8_in[:, :, nsl])
                    # stream remaining qlp as s-pairs, two j-chunks ahead
                    u0 = 8 + 4 * jc
                    if u0 < NS * 2:
                        nc.sync.dma_start(
                            qlb[:, u0:u0 + 4, :],
                            qlp8_in[:, u0 * CP:(u0 + 4) * CP])
                    # pq: sim8[b-chunk, j-chunk], one DoubleRow matmul
                    # each; ship in groups of 4 j-chunks. The final group is
                    # split into two half-transfers so it lands before the
                    # last matmuls finish.
                    for bt in range(4):
                        pq = psm.tile([128, 512], F32, name="pq", tag="pq")
                        nc.tensor.matmul(
                            pq[:], qT8[:, :, bt * 128:(bt + 1) * 128],
                            qsh8[:, :, jsl], start=True, stop=True,
                            perf_mode=PM.DoubleRow)
                        if jc % 4 == 0:
                            st_cur[bt] = sq.tile([128, 2048], FP8,
                                                 name=f"st{bt}", tag=f"st{bt}")
                        st = st_cur[bt]
                        nc.vector.tensor_copy(
                            st[:, (jc % 4) * 512:(jc % 4) * 512 + 512], pq[:])
                        if jc % 4 == 3 and jc < NJC - 1:
                            nc.sync.dma_start(
                                simw_out[bt * 128:(bt + 1) * 128,
                                         (jc - 3) * 512:(jc + 1) * 512],
                                st[:])
                        elif jc == NJC - 1:
                            half = (jc - 3 + 2) * 512
                            nc.sync.dma_start(
                                simw_out[bt * 128:(bt + 1) * 128,
                                         (jc - 3) * 512:half],
                                st[:, 0:1024])
                            nc.sync.dma_start(
                                simw_out[bt * 128:(bt + 1) * 128,
                                         half:(jc + 1) * 512],
                                st[:, 1024:2048])
                    # pk + exp: ET tiles for 4 j-tiles
                    for tt in range(4):
                        t = 4 * jc + tt
                        pk = psm.tile([128, 512], F32, name="pk", tag="pk")
                        nc.tensor.matmul(
                            pk[:], qsh8[:, :, t * 128:(t + 1) * 128], kT8[:],
                            start=True, stop=True, perf_mode=PM.DoubleRow)
                        nc.scalar.activation(et3[:, t, :], pk[:], Act.Exp,
                                             scale=1.0 / T_DC)
                    # phase-2 pass A (one jc behind so qlb/ET are ready):
                    # accumulate P for b-tiles 0,1
                    if jc > 0:
                        _p2a(nc, pacc, et3, qlb, jc - 1)
                _p2a(nc, pacc, et3, qlb, NJC - 1)
            # phase-2 pass B: b-tiles 2,3 (et + qlp fully resident)
            with ExitStack() as ph2:
                psB = ph2.enter_context(
                    tc.tile_pool(name="psB", bufs=1, space="PSUM"))
                paccB = [psB.tile([128, CP], F32, name=f"paccB{i}")
                         for i in range(2)]
                # drain pass A results while pass B streams
                for bt in range(2):
                    pcp = sq.tile([128, CP], F32, name="pcp", tag="pcp")
                    nc.vector.tensor_copy(pcp[:], pacc[bt][:])
                    for q in range(4):
                        nc.sync.dma_start(
                            p_out[bt * 128:(bt + 1) * 128,
                                  q * 256:(q + 1) * 256],
                            pcp[:, q * 256:(q + 1) * 256])
                # pass B as four (cs, bt) sweeps; each drains while the
                # next sweep streams
                for sw, (cs, bt) in enumerate(
                        ((0, 0), (0, 1), (1, 0), (1, 1))):
                    c0, c1 = (0, 512) if cs == 0 else (512, 1001)
                    for s in range(NS):
                        nc.tensor.matmul(
                            paccB[bt][:, c0:c1],
                            et3[:, 2 * s:2 * s + 2,
                                (bt + 2) * 128:(bt + 3) * 128],
                            qlb[:, 2 * s:2 * s + 2, c0:c1],
                            start=(s == 0), stop=(s == NS - 1),
                            perf_mode=PM.DoubleRow)
                    pcp = sq.tile([128, c1 - c0], F32, name=f"pcpB{sw}",
                                  tag="pcpB")
                    if sw % 2 == 0:
                        nc.vector.tensor_copy(pcp[:], paccB[bt][:, c0:c1])
                    else:
                        nc.scalar.activation(pcp[:], paccB[bt][:, c0:c1],
                                             Act.Copy)
                    w = c1 - c0
                    qw = w // 4
                    cuts = [0, qw, 2 * qw, 3 * qw, w]
                    for q in range(4):
                        a0, a1 = cuts[q], cuts[q + 1]
                        eng = nc.sync if (sw + q) % 2 == 0 else nc.scalar
                        eng.dma_start(
                            p_out[(bt + 2) * 128:(bt + 3) * 128,
                                  c0 + a0:c0 + a1],
                            pcp[:, a0:a1])

    _CACHED['nc'] = nc
    return nc


def _fp8_sr(x, rng):
    """Stochastic-round non-negative float64 array to fp8 e4m3 (unbiased)."""
    a8 = x.astype(FP8NP)
    au = a8.view(np.uint8)
    af = a8.astype(np.float64)
    hi_u = np.where(af < x, au + 1, au).astype(np.uint8)
    lo_u = np.where(af > x, au - 1, au).astype(np.uint8)
    lof = lo_u.view(FP8NP).astype(np.float64)
    hif = hi_u.view(FP8NP).astype(np.float64)
    d = hif - lof
    p = np.where(d > 0, (x - lof) / np.where(d == 0, 1.0, d), 0.0)
    u = rng.random(x.shape)
    return np.where(u < p, hi_u, lo_u).view(FP8NP)


def _in_maps(norm_q, k_feat, queue, qlp):
    """Build per-core input maps (host-side layout shuffles + casts)."""
    def dhb(mat_t, dtype):
        # [256, N] -> [128 d, 2 h, N] with row = h*128+d
        return np.ascontiguousarray(
            mat_t.reshape(2, 128, -1).transpose(1, 0, 2)).astype(dtype)

    qT8 = dhb(np.ascontiguousarray(norm_q.T), FP8NP)
    kT8 = dhb(np.ascontiguousarray(k_feat.T), FP8NP)
    rng = np.random.default_rng(1234)
    in_maps = []
    for c in range(N_CORES):
        sh = slice(c * KS, (c + 1) * KS)
        qsh = np.ascontiguousarray(queue[:, sh])
        qlp_aug = np.zeros((KS, CP), np.float64)
        qlp_aug[:, :C] = qlp[:, sh].T.astype(np.float64) * QLP_SCALE
        qlp_aug[:, C] = QLP_SCALE
        qlp8 = _fp8_sr(qlp_aug, rng)
        # [j, c] -> [p, s*2+h, c] -> [128, NS*2*CP]; j = (s*2+h)*128 + p
        qlp8_p = qlp8.reshape(NS * 2, 128, CP).transpose(1, 0, 2)
        in_maps.append({
            "qT8": qT8, "kT8": kT8,
            "qsh8": dhb(qsh, FP8NP),
            "qlp8": np.ascontiguousarray(qlp8_p).reshape(128, NS * 2 * CP),
        })
    return in_maps


def kernel(norm_q, q_logits, k_feat, logits_k, queue, queue_label_prob,
           queue_label, target, knn_k):
    norm_q = np.asarray(norm_q, np.float32)
    q_logits = np.asarray(q_logits, np.float32)
    k_feat = np.asarray(k_feat, np.float32)
    queue = np.asarray(queue, np.float32)
    qlp = np.asarray(queue_label_prob, np.float32)
    queue_label = np.asarray(queue_label)
    target = np.asarray(target)
    kk = int(knn_k)

    nc = _build()
    in_maps = _in_maps(norm_q, k_feat, queue, qlp)
    res = run_bass_kernel_spmd(nc, in_maps, list(range(N_CORES)))

    sim8 = np.concatenate(
        [res.results[c]["simw"].astype(np.float32) for c in range(N_CORES)],
        axis=1)
    P = np.zeros((B, CP), np.float64)
    for c in range(N_CORES):
        P += res.results[c]["pout"].astype(np.float64)

    # ---- supcon: fp8 candidates -> exact fp64 refine -> exact top-k ----
    M = max(CAND, 2 * kk)
    cand = np.argpartition(-sim8, M - 1, axis=1)[:, :M]
    nq64 = norm_q.astype(np.float64)
    qu64T = queue.T.astype(np.float64)         # [K, D]
    sim_cand = np.empty((B, M))
    step = 128
    for r0 in range(0, B, step):
        r1 = min(r0 + step, B)
        qg = qu64T[cand[r0:r1]]                # [rows, M, D]
        sim_cand[r0:r1] = np.einsum('rd,rmd->rm', nq64[r0:r1], qg)
    sel = np.argpartition(-sim_cand, kk - 1, axis=1)[:, :kk]
    idx = np.take_along_axis(cand, sel, axis=1)
    sim_knn = np.take_along_axis(sim_cand, sel, axis=1)
    w = np.exp((sim_knn - sim_knn.max(axis=1, keepdims=True)) / T_SUP)
    w /= w.sum(axis=1, keepdims=True)
    pos = (target[:, None] == queue_label[idx])
    gt = (w * pos).sum(axis=1)
    m = gt > EPS
    supin_loss = np.where(m, -np.log(np.where(m, gt, 1.0)), 0.0).sum() / B

    # ---- fc loss ----
    x = q_logits.astype(np.float64)
    lse = np.log(np.exp(x - x.max(1, keepdims=True)).sum(1)) + x.max(1)
    log_q = x - lse[:, None]
    q_mask = (x.min(1) - lse) > np.log(EPS)
    onehot = np.full((B, C), LS / (C - 1))
    onehot[np.arange(B), target] = 1.0 - LS
    fc_loss = -((onehot * log_q).sum(1) * q_mask).sum() / B

    # ---- dc loss ----
    Z = P[:, C]
    dc_t = P[:, :C] / Z[:, None]
    dc_pos = dc_t > 0
    kl = np.where(dc_pos,
                  dc_t * (np.log(np.where(dc_pos, dc_t, 1.0)) - log_q), 0.0)
    dc_loss = (kl.sum(1) * q_mask).sum() / B

    return (np.float32(supin_loss), np.float32(fc_loss), np.float32(dc_loss))


# revision 24
# speedup vs baseline: 1.0175x; 1.0175x over previous
"""Trainium2 Bass kernel for nn_CoNe_35974646071945 (retrieval_knn).

Strategy: K-shard the 65536-entry queue across 8 NeuronCores.
Per core (shard of KS=8192 queue columns):
  pq:  sim8[b, j] = norm_q @ queue_shard as one fp8 DoubleRow matmul per
       (b-tile, j-chunk) (256-deep contraction per pass), shipped fp8.
       The host then refines: exact fp64 sims for the top-1024 fp8
       candidates per row (the true top-200 sit within fp8-rank ~570 on
       this distribution; 1024 gives 2x margin), so the top-200
       selection and softmax weights are exact.
  pk:  simk^T[j, b] via one fp8 DoubleRow matmul per 128-j tile, then
       ET = exp(10*simk) on the scalar engine, stored fp8 in SBUF.
  p2:  P[b, c'] = ET^T @ qlp_aug (fp8 DoubleRow, 64x-scaled
       stochastically-rounded qlp so the fp8 rounding is unbiased;
       col 1000 = 64 gives the softmax partition Z). PSUM groups are
       512-column (bank-aligned) so the bank-granular start-zeroing
       cannot clobber a sibling accumulation group.
Host: exact top-200/softmax for supcon, fc loss, and the KL with
dc_t = P[:, :1000]/P[:, 1000] (the 64x scale cancels).
"""
import sys
sys.path.insert(0, '/opt/trn_rl_repo')
sys.path.insert(0, '/root/.axon_site/_ro/trn_rl_repo')

import numpy as np
import ml_dtypes
from contextlib import ExitStack

from concourse import bass, tile, mybir
from concourse.bass_utils import run_bass_kernel_spmd
from concourse.vector_clock import ScopedClock, VectorClock

F32 = mybir.dt.float32
F16 = mybir.dt.float16
FP8 = mybir.dt.float8e4
Act = mybir.ActivationFunctionType
PM = mybir.MatmulPerfMode
FP8NP = ml_dtypes.float8_e4m3

N_CORES = 8
B, D, K, C = 512, 256, 65536, 1000
KS = K // N_CORES            # 8192 queue columns per core
T_SUP, T_DC, LS = 0.07, 0.1, 0.1
EPS = 1e-8
NJT = KS // 128              # 64 j-tiles per core
NS = NJT // 2                # 32 double-j-tiles (fp8 DoubleRow)
NJC = KS // 512              # 16 j-chunks for pq
CP = 1024                    # padded class dim (1000 probs + Z col + pad)
QLP_SCALE = 64.0
CAND = 1024                  # fp8 candidates refined per row on host


class CompatTileContext(tile.TileContext):
    """This walrus build encodes at most ONE sync wait per instruction.
    Split Tile's multi-wait instructions and its tail drain."""

    def _commit_instruction(self, inst, lazy_reg_writes=True):
        si = inst.sync_info
        if (
            si is not None
            and si.on_wait
            and len(si.on_wait) > 1
            and inst.engine != mybir.EngineType.Unassigned
        ):
            import bass_rust
            waits = list(si.on_wait)
            for w in waits[:-1]:
                nop = mybir.InstNoOp(
                    name=f"I-{self.nc.next_id()}", ins=[], outs=[]
                )
                nop.engine = inst.engine
                nop.sync_info = bass_rust.SyncInfo(on_wait=[w], on_update=[])
                super()._commit_instruction(nop, lazy_reg_writes=False)
            si.on_wait = [waits[-1]]
            inst.sync_info = si
        super()._commit_instruction(inst, lazy_reg_writes=lazy_reg_writes)

    def _drain_and_barrier(self, tick_clock, wait_clock):
        gclock = tick_clock.global_clock
        n = len(gclock)
        for i in range(n):
            if gclock[i] == 0:
                continue
            vec = [0] * n
            vec[i] = gclock[i]
            nop_inst = self.nc.sync.nop(nofuse=True, hint=f"tail_wait_p{i}")
            wait_clock.add_sem_waits(
                nop_inst.ins, ScopedClock({None: VectorClock(vec)})
            )
        self.nc.sync.drain()
        self.nc.all_engine_barrier()
        assert self.sems is not None
        popped = self.nc._tile_sem_poison_stack.pop()
        assert popped is self._sem_poison
        self.nc.clear_and_free_semaphores(list(self.sems.allocated().values()))
        self.nc.all_engine_barrier()


_CACHED = {}


def _p2a(nc, pacc, et3, qlb, jc):
    """Phase-2 pass A matmuls for the s-pair of j-chunk jc (b-tiles 0,1)."""
    for si in range(2):
        s = 2 * jc + si
        for bt in range(2):
            lhs = et3[:, 2 * s:2 * s + 2, bt * 128:(bt + 1) * 128]
            for c0, c1 in ((0, 512), (512, 1001)):
                nc.tensor.matmul(
                    pacc[bt][:, c0:c1],
                    lhs,
                    qlb[:, 2 * s:2 * s + 2, c0:c1],
                    start=(s == 0), stop=(s == NS - 1),
                    perf_mode=PM.DoubleRow)


def _build():
    if 'nc' in _CACHED:
        return _CACHED['nc']
    nc = bass.Bass(num_devices=N_CORES)
    qT8_in = nc.declare_dram_parameter("qT8", [128, 2, B], FP8, isOutput=False)
    kT8_in = nc.declare_dram_parameter("kT8", [128, 2, B], FP8, isOutput=False)
    qsh8_in = nc.declare_dram_parameter("qsh8", [128, 2, KS], FP8, isOutput=False)
    # host layout [p][s][h][c] flattened to [128, NS*2*CP]
    qlp8_in = nc.declare_dram_parameter("qlp8", [128, NS * 2 * CP], FP8,
                                        isOutput=False)
    simw_out = nc.declare_dram_parameter("simw", [B, KS], FP8, isOutput=True)
    p_out = nc.declare_dram_parameter("pout", [B, CP], F32, isOutput=True)

    with ExitStack() as ctx:
        tc = ctx.enter_context(CompatTileContext(nc))
        pool = ctx.enter_context(tc.tile_pool(name="main", bufs=1))
        sq = ctx.enter_context(tc.tile_pool(name="sq", bufs=2))

        qT8 = pool.tile([128, 2, B], FP8, name="qT8")
        kT8 = pool.tile([128, 2, B], FP8, name="kT8")
        qsh8 = pool.tile([128, 2, KS], FP8, name="qsh8")
        et3 = pool.tile([128, NJT, B], FP8, name="et3")
        qlb = pool.tile([128, NS * 2, CP], FP8, name="qlb")

        # head: h-granular so the first pq/pk matmuls start ASAP
        for h in range(2):
            nc.sync.dma_start(qT8[:, h, :], qT8_in[:, h, :])
            nc.sync.dma_start(qsh8[:, h, 0:512], qsh8_in[:, h, 0:512])
        nc.sync.dma_start(qsh8[:, :, 512:1024], qsh8_in[:, :, 512:1024])
        for h in range(2):
            nc.sync.dma_start(kT8[:, h, :], kT8_in[:, h, :])
        for u in range(8):
            nc.sync.dma_start(qlb[:, u:u + 1, :],
                              qlp8_in[:, u * CP:(u + 1) * CP])

        warm8 = pool.tile([128, 512], FP8, name="warm8")
        nc.vector.memset(warm8[:], 0.0)

        with ExitStack() as ph:
            psA = ph.enter_context(tc.tile_pool(name="psA", bufs=1, space="PSUM"))
            pacc = [psA.tile([128, CP], F32, name=f"pacc{i}") for i in range(2)]
            with ExitStack() as ph1:
                psm = ph1.enter_context(
                    tc.tile_pool(name="psm", bufs=2, space="PSUM"))
                st_cur = {}
                # ramp the PE to full p-state while the head DMAs land
                for _ in range(10):
                    wp = psm.tile([128, 512], F32, name="wp", tag="pk")
                    nc.tensor.matmul(wp[0:64, :], warm8[:, 0:64], warm8[:],
                                     start=True, stop=True)
                for jc in range(NJC):
                    jsl = slice(jc * 512, (jc + 1) * 512)
                    if jc + 2 < NJC:
                        nsl = slice((jc + 2) * 512, (jc + 3) * 512)
                        nc.sync.dma_start(qsh8[:, :, nsl], qsh8_in[:, :, nsl])
                    # stream remaining qlp as s-pairs, two j-chunks ahead
                    u0 = 8 + 4 * jc
                    if u0 < NS * 2:
                        nc.sync.dma_start(
                            qlb[:, u0:u0 + 4, :],
                            qlp8_in[:, u0 * CP:(u0 + 4) * CP])
                    # pq: sim8[b-chunk, j-chunk], one DoubleRow matmul
                    # each; ship in groups of 4 j-chunks. The final group is
                    # split into two half-transfers so it lands before the
                    # last matmuls finish.
                    for bt in range(4):
                        pq = psm.tile([128, 512], F32, name="pq", tag="pq")
                        nc.tensor.matmul(
                            pq[:], qT8[:, :, bt * 128:(bt + 1) * 128],
                            qsh8[:, :, jsl], start=True, stop=True,
                            perf_mode=PM.DoubleRow)
                        if jc % 4 == 0:
                            st_cur[bt] = sq.tile([128, 2048], FP8,
                                                 name=f"st{bt}", tag=f"st{bt}")
                        st = st_cur[bt]
                        nc.vector.tensor_copy(
                            st[:, (jc % 4) * 512:(jc % 4) * 512 + 512], pq[:])
                        if jc % 4 == 3 and jc < NJC - 1:
                            nc.sync.dma_start(
                                simw_out[bt * 128:(bt + 1) * 128,
                                         (jc - 3) * 512:(jc + 1) * 512],
                                st[:])
                        elif jc == NJC - 1:
                            half = (jc - 3 + 2) * 512
                            nc.sync.dma_start(
                                simw_out[bt * 128:(bt + 1) * 128,
                                         (jc - 3) * 512:half],
                                st[:, 0:1024])
                            nc.sync.dma_start(
                                simw_out[bt * 128:(bt + 1) * 128,
                                         half:(jc + 1) * 512],
                                st[:, 1024:2048])
                    # pk + exp: ET tiles for 4 j-tiles
                    for tt in range(4):
                        t = 4 * jc + tt
                        pk = psm.tile([128, 512], F32, name="pk", tag="pk")
                        nc.tensor.matmul(
                            pk[:], qsh8[:, :, t * 128:(t + 1) * 128], kT8[:],
                            start=True, stop=True, perf_mode=PM.DoubleRow)
                        nc.scalar.activation(et3[:, t, :], pk[:], Act.Exp,
                                             scale=1.0 / T_DC)
                    # phase-2 pass A (one jc behind so qlb/ET are ready):
                    # accumulate P for b-tiles 0,1
                    if jc > 0:
                        _p2a(nc, pacc, et3, qlb, jc - 1)
                _p2a(nc, pacc, et3, qlb, NJC - 1)
            # phase-2 pass B: b-tiles 2,3 (et + qlp fully resident)
            with ExitStack() as ph2:
                psB = ph2.enter_context(
                    tc.tile_pool(name="psB", bufs=1, space="PSUM"))
                paccB = [psB.tile([128, CP], F32, name=f"paccB{i}")
                         for i in range(2)]
                # drain pass A results while pass B streams
                for bt in range(2):
                    pcp = sq.tile([128, CP], F32, name="pcp", tag="pcp")
                    nc.vector.tensor_copy(pcp[:], pacc[bt][:])
                    for q in range(4):
                        nc.sync.dma_start(
                            p_out[bt * 128:(bt + 1) * 128,
                                  q * 256:(q + 1) * 256],
                            pcp[:, q * 256:(q + 1) * 256])
                # pass B as four (cs, bt) sweeps; each drains while the
                # next sweep streams
                for sw, (cs, bt) in enumerate(
                        ((0, 0), (0, 1), (1, 0), (1, 1))):
                    c0, c1 = (0, 512) if cs == 0 else (512, 1001)
                    for s in range(NS):
                        nc.tensor.matmul(
                            paccB[bt][:, c0:c1],
                            et3[:, 2 * s:2 * s + 2,
                                (bt + 2) * 128:(bt + 3) * 128],
                            qlb[:, 2 * s:2 * s + 2, c0:c1],
                            start=(s == 0), stop=(s == NS - 1),
                            perf_mode=PM.DoubleRow)
                    pcp = sq.tile([128, c1 - c0], F32, name=f"pcpB{sw}",
                                  tag="pcpB")
                    if sw % 2 == 0:
                        nc.vector.tensor_copy(pcp[:], paccB[bt][:, c0:c1])
                    else:
                        nc.scalar.activation(pcp[:], paccB[bt][:, c0:c1],
                                             Act.Copy)
                    w = c1 - c0
                    mid = w // 2
                    for q, (a0, a1) in enumerate(((0, mid), (mid, w))):
                        eng = nc.sync if (sw + q) % 2 == 0 else nc.scalar
                        eng.dma_start(
                            p_out[(bt + 2) * 128:(bt + 3) * 128,
                                  c0 + a0:c0 + a1],
                            pcp[:, a0:a1])

    _CACHED['nc'] = nc
    return nc


def _fp8_sr(x, rng):
    """Stochastic-round non-negative float64 array to fp8 e4m3 (unbiased)."""
    a8 = x.astype(FP8NP)
    au = a8.view(np.uint8)
    af = a8.astype(np.float64)
    hi_u = np.where(af < x, au + 1, au).astype(np.uint8)
    lo_u = np.where(af > x, au - 1, au).astype(np.uint8)
    lof = lo_u.view(FP8NP).astype(np.float64)
    hif = hi_u.view(FP8NP).astype(np.float64)
    d = hif - lof
    p = np.where(d > 0, (x - lof) / np.where(d == 0, 1.0, d), 0.0)
    u = rng.random(x.shape)
    return np.where(u < p, hi_u, lo_u).view(FP8NP)


def _in_maps(norm_q, k_feat, queue, qlp):
    """Build per-core input maps (host-side layout shuffles + casts)."""
    def dhb(mat_t, dtype):
        # [256, N] -> [128 d, 2 h, N] with row = h*128+d
        return np.ascontiguousarray(
            mat_t.reshape(2, 128, -1).transpose(1, 0, 2)).astype(dtype)

    qT8 = dhb(np.ascontiguousarray(norm_q.T), FP8NP)
    kT8 = dhb(np.ascontiguousarray(k_feat.T), FP8NP)
    rng = np.random.default_rng(1234)
    in_maps = []
    for c in range(N_CORES):
        sh = slice(c * KS, (c + 1) * KS)
        qsh = np.ascontiguousarray(queue[:, sh])
        qlp_aug = np.zeros((KS, CP), np.float64)
        qlp_aug[:, :C] = qlp[:, sh].T.astype(np.float64) * QLP_SCALE
        qlp_aug[:, C] = QLP_SCALE
        qlp8 = _fp8_sr(qlp_aug, rng)
        # [j, c] -> [p, s*2+h, c] -> [128, NS*2*CP]; j = (s*2+h)*128 + p
        qlp8_p = qlp8.reshape(NS * 2, 128, CP).transpose(1, 0, 2)
        in_maps.append({
            "qT8": qT8, "kT8": kT8,
            "qsh8": dhb(qsh, FP8NP),
            "qlp8": np.ascontiguousarray(qlp8_p).reshape(128, NS * 2 * CP),
        })
    return in_maps


def kernel(norm_q, q_logits, k_feat, logits_k, queue, queue_label_prob,
           queue_label, target, knn_k):
    norm_q = np.asarray(norm_q, np.float32)
    q_logits = np.asarray(q_logits, np.float32)
    k_feat = np.asarray(k_feat, np.float32)
    queue = np.asarray(queue, np.float32)
    qlp = np.asarray(queue_label_prob, np.float32)
    queue_label = np.asarray(queue_label)
    target = np.asarray(target)
    kk = int(knn_k)

    nc = _build()
    in_maps = _in_maps(norm_q, k_feat, queue, qlp)
    res = run_bass_kernel_spmd(nc, in_maps, list(range(N_CORES)))

    sim8 = np.concatenate(
        [res.results[c]["simw"].astype(np.float32) for c in range(N_CORES)],
        axis=1)
    P = np.zeros((B, CP), np.float64)
    for c in range(N_CORES):
        P += res.results[c]["pout"].astype(np.float64)

    # ---- supcon: fp8 candidates -> exact fp64 refine -> exact top-k ----
    M = max(CAND, 2 * kk)
    cand = np.argpartition(-sim8, M - 1, axis=1)[:, :M]
    nq64 = norm_q.astype(np.float64)
    qu64T = queue.T.astype(np.float64)         # [K, D]
    sim_cand = np.empty((B, M))
    step = 128
    for r0 in range(0, B, step):
        r1 = min(r0 + step, B)
        qg = qu64T[cand[r0:r1]]                # [rows, M, D]
        sim_cand[r0:r1] = np.einsum('rd,rmd->rm', nq64[r0:r1], qg)
    sel = np.argpartition(-sim_cand, kk - 1, axis=1)[:, :kk]
    idx = np.take_along_axis(cand, sel, axis=1)
    sim_knn = np.take_along_axis(sim_cand, sel, axis=1)
    w = np.exp((sim_knn - sim_knn.max(axis=1, keepdims=True)) / T_SUP)
    w /= w.sum(axis=1, keepdims=True)
    pos = (target[:, None] == queue_label[idx])
    gt = (w * pos).sum(axis=1)
    m = gt > EPS
    supin_loss = np.where(m, -np.log(np.where(m, gt, 1.0)), 0.0).sum() / B

    # ---- fc loss ----
    x = q_logits.astype(np.float64)
    lse = np.log(np.exp(x - x.max(1, keepdims=True)).sum(1)) + x.max(1)
    log_q = x - lse[:, None]
    q_mask = (x.min(1) - lse) > np.log(EPS)
    onehot = np.full((B, C), LS / (C - 1))
    onehot[np.arange(B), target] = 1.0 - LS
    fc_loss = -((onehot * log_q).sum(1) * q_mask).sum() / B

    # ---- dc loss ----
    Z = P[:, C]
    dc_t = P[:, :C] / Z[:, None]
    dc_pos = dc_t > 0
    kl = np.where(dc_pos,
                  dc_t * (np.log(np.where(dc_pos, dc_t, 1.0)) - log_q), 0.0)
    dc_loss = (kl.sum(1) * q_mask).sum() / B

    return (np.float32(supin_loss), np.float32(fc_loss), np.float32(dc_loss))
